# revision 1
# baseline (speedup 1.0000x reference)
"""SIR-MLP RK4 scan kernel for 8 Trainium2 cores.

Math (per batch element b):
  5 small MLPs produce params R, gamma, I0, kint, tint.
  beta(t) = sigmoid((t - tint)/1.75) * (beta1 - beta0) + beta0,
  beta0 = R*gamma, beta1 = beta0*kint.
  RK4 with dt=1 over t = 0..364 of SIR:
     dS/dt = -beta*S*I/N,  dI/dt = beta*S*I/N - gamma*I
  output[t, b] = S[t-1, b] - S[t, b]  (0 for t=0), reshaped (-1, 365).

Device strategy (pure data parallel, batch sharded 8 ways, 2048/core):
  - state held negated (Sh = -S) so k_S = +flux; flux = qneg * (Sh * I)
    with qneg = -beta/N from a precomputed [128, 729*16] table.
  - RK4 scan runs entirely on the vector engine: the walrus build in this
    toolchain accepts at most ONE sync wait per engine instruction, so the
    hot loop must be single-engine (program order only).  Phase hand-offs
    use strict all-engine barriers (the barrier nop is a SEQ instruction
    and may carry many waits).
  - batch layout on a core: column b in [0,2048) of xT; scan tiles are
    [128 partitions, 16 cols] with b = j*128 + p  (j = col, p = partition),
    which lets the MLP head write params directly into scan layout via
    one [32,128]-stationary matmul per (net, j).
  - dI history accumulates in a [128, 16*365] SBUF buffer, one contiguous
    3 MB DMA to DRAM at the end; host un-permutes rows.
"""

import os
import numpy as np

B = 16384
D_IN = 8
H = 32
T = 365
NN = 5
NH = 3
N_CORES = 8
BL = B // N_CORES          # 2048 batch per core
FD = BL // 128             # 16 batch columns per partition
N_POP = 8.6e6
GATE = 7.0 / 4.0
NSTEP = T - 1
NGRID = 2 * NSTEP + 1      # 729 half-step time points

# packed-weights column offsets in wpack [128, WPK]
OFF_W0A = 0              # [8, 128]  W0 nets 0-3, d-major
OFF_W0B = 128            # [8, 32]   W0 net 4
OFF_WHA = 160            # + 160*l   [128, 128] block-diag Wh nets 0-3
OFF_WHB = 288            # + 160*l   [32, 32]   Wh net 4
OFF_WO = 640             # [128, 4]  Wo nets 0-3 block rows
OFF_WOB = 644            # [32, 1]   Wo net 4
OFF_B0A = 645            # [128, 1]
OFF_B0B = 646            # [32, 1]
OFF_BHA = 647            # + 2*l [128, 1]
OFF_BHB = 648            # + 2*l [32, 1]
OFF_BOT = 653            # [128, 80] bo replicated in head layout
OFF_WO3 = 733            # [32, 1] Wo net 3 at base partition 0
OFF_X = 734              # [8, BL]
WPK = 734 + BL

_CACHE = {}


def _build_program(n_steps):
    import concourse.bass as bass
    import concourse.tile as tile
    import concourse.mybir as mybir
    from concourse.bass import _add_dep_helper

    dt_f32 = mybir.dt.float32
    op = mybir.AluOpType
    act = mybir.ActivationFunctionType
    ngrid = 2 * n_steps + 1
    nt = n_steps + 1

    nc = bass.Bass()

    tg = nc.declare_dram_parameter("tgridG", [128, ngrid], dt_f32, isOutput=False)
    wpack = nc.declare_dram_parameter("wpack", [128, WPK], dt_f32, isOutput=False)
    dI_out = nc.declare_dram_parameter("dI", [BL, nt], dt_f32, isOutput=True)

    with tile.TileContext(nc) as tc:  # noqa: SIM117
        with (
            tc.tile_pool(name="big", bufs=1) as big,
            tc.tile_pool(name="mlp", bufs=2) as mlp,
            tc.tile_pool(name="wts", bufs=1) as wts,
            tc.tile_pool(name="small", bufs=1) as small,
            tc.tile_pool(name="psA", bufs=2, space="PSUM") as psA,
        ):
            # ---------------- loads ----------------
            wp = wts.tile([128, WPK], dt_f32, tag="wp")
            nc.sync.dma_start(wp[:], wpack[:])
            tgrid_sb = big.tile([128, ngrid], dt_f32, tag="tgrid")
            nc.sync.dma_start(tgrid_sb[:], tg[:])

            x_sb = wp[0:D_IN, OFF_X:OFF_X + BL]
            w0A_sb = wp[0:D_IN, OFF_W0A:OFF_W0A + 128]
            w0B_sb = wp[0:D_IN, OFF_W0B:OFF_W0B + H]
            whA_sb = [wp[:, OFF_WHA + 160 * l:OFF_WHA + 160 * l + 128]
                      for l in range(NH)]
            whB_sb = [wp[0:H, OFF_WHB + 160 * l:OFF_WHB + 160 * l + H]
                      for l in range(NH)]
            wo_sb = wp[:, OFF_WO:OFF_WO + 4]
            woB_sb = wp[0:H, OFF_WOB:OFF_WOB + 1]
            b0A_sb = wp[:, OFF_B0A:OFF_B0A + 1]
            b0B_sb = wp[0:H, OFF_B0B:OFF_B0B + 1]
            bhA_sb = [wp[:, OFF_BHA + 2 * l:OFF_BHA + 2 * l + 1] for l in range(NH)]
            bhB_sb = [wp[0:H, OFF_BHB + 2 * l:OFF_BHB + 2 * l + 1] for l in range(NH)]
            boT_sb = wp[:, OFF_BOT:OFF_BOT + 5 * FD]
            wo3_sb = wp[0:H, OFF_WO3:OFF_WO3 + 1]

            # pre-touch scratches: each op below carries exactly ONE wait,
            # advancing its engine's observed clock for one other processor
            pt = small.tile([128, 6], dt_f32, tag="pt")
            ptv = small.tile([128, 4], dt_f32, tag="ptv")
            bf16 = mybir.dt.bfloat16
            nc.scalar.activation(pt[:, 0:1], wp[:, 0:1], act.Copy,
                                 bias=0.0, scale=1.0)
            nc.scalar.activation(pt[:, 1:2], tgrid_sb[:, 0:1], act.Copy,
                                 bias=0.0, scale=1.0)
            nc.vector.tensor_copy(ptv[:, 0:1], wp[:, 0:1])
            hist = big.tile([128, FD * (n_steps + 1)], dt_f32, tag="hist")
            nc.vector.memset(hist[:], 0.0)
            histv = hist[:].rearrange("p (j t) -> p j t", t=n_steps + 1)

            # ---------------- MLP ----------------
            # nets 0-3 packed on 128 partitions; net 4 in [H, BL] tiles
            # PSUM: psmlp [128,1024]x2 (4 banks) + psB [32,1024]x2 (4 banks)
            h_prev = mlp.tile([128, BL], dt_f32, tag="h")
            hB_prev = mlp.tile([H, BL], dt_f32, tag="hB")
            for half in range(2):
                hs = slice(half * 1024, (half + 1) * 1024)
                ps = psA.tile([128, 1024], dt_f32, tag="psmlp")
                psB = psA.tile([H, 1024], dt_f32, tag="psB")
                for c in range(2):
                    sl = slice(half * 1024 + c * 512, half * 1024 + (c + 1) * 512)
                    pl = slice(c * 512, (c + 1) * 512)
                    nc.tensor.matmul(ps[:, pl], w0A_sb, x_sb[:, sl])
                    nc.tensor.matmul(psB[:, pl], w0B_sb, x_sb[:, sl])
                nc.scalar.activation(h_prev[:, hs], ps[:], act.Tanh,
                                     bias=b0A_sb, scale=1.0)
                nc.scalar.activation(hB_prev[:, hs], psB[:], act.Tanh,
                                     bias=b0B_sb, scale=1.0)

            for l in range(NH):
                h_new = mlp.tile([128, BL], dt_f32, tag="h")
                hB_new = mlp.tile([H, BL], dt_f32, tag="hB")
                for half in range(2):
                    hs = slice(half * 1024, (half + 1) * 1024)
                    psl = psA.tile([128, 1024], dt_f32, tag="psmlp")
                    pslB = psA.tile([H, 1024], dt_f32, tag="psB")
                    # PE pre-touch: bf16-bitcast ldweights reads h (1 Act wait)
                    nc.tensor.ldweights(
                        h_prev[0:1, half * 1024:half * 1024 + 2].bitcast(bf16))
                    nc.tensor.ldweights(
                        hB_prev[0:1, half * 1024:half * 1024 + 2].bitcast(bf16))
                    for c in range(2):
                        sl = slice(half * 1024 + c * 512,
                                   half * 1024 + (c + 1) * 512)
                        pl = slice(c * 512, (c + 1) * 512)
                        nc.tensor.matmul(psl[:, pl], whA_sb[l], h_prev[:, sl])
                        nc.tensor.matmul(pslB[:, pl], whB_sb[l], hB_prev[:, sl])
                    # ACT pre-touch of the psum tiles before the tanhs
                    nc.scalar.activation(pt[:, 2:3], psl[0:128, 0:1], act.Copy,
                                         bias=0.0, scale=1.0)
                    nc.scalar.activation(pt[:, 3:4][0:H], pslB[0:H, 0:1],
                                         act.Copy, bias=0.0, scale=1.0)
                    nc.scalar.activation(h_new[:, hs], psl[:], act.Tanh,
                                         bias=bhA_sb[l], scale=1.0)
                    nc.scalar.activation(hB_new[:, hs], pslB[:], act.Tanh,
                                         bias=bhB_sb[l], scale=1.0)
                h_prev, hB_prev = h_new, hB_new

            # ------------- head: params straight into scan layout -------------
            # psO[:, n*FD + j] = Wo[n] . h[net n, batch col block j]
            # (one [32,128]-stationary matmul per (net, j); b = j*128 + p)
            psO = psA.tile([128, 1024], dt_f32, tag="psmlp")
            # PE operands may only start at partition 0/32/64: net 3 (base 96)
            # is copied down to a base-0 tile first
            h3 = mlp.tile([H, BL], dt_f32, tag="h3")
            nc.vector.tensor_copy(h3[:], h_prev[96:128, :])
            nc.tensor.ldweights(h_prev[0:1, 0:2].bitcast(bf16))
            nc.tensor.ldweights(hB_prev[0:1, 0:2].bitcast(bf16))
            nc.tensor.ldweights(h3[0:1, 0:2].bitcast(bf16))
            for j in range(FD):
                bsl = slice(j * 128, (j + 1) * 128)
                for n in range(3):
                    nc.tensor.matmul(psO[:, n * FD + j:n * FD + j + 1],
                                     h_prev[n * H:(n + 1) * H, bsl],
                                     wo_sb[n * H:(n + 1) * H, n:n + 1])
                nc.tensor.matmul(psO[:, 3 * FD + j:3 * FD + j + 1],
                                 h3[:, bsl], wo3_sb)
                nc.tensor.matmul(psO[:, 4 * FD + j:4 * FD + j + 1],
                                 hB_prev[:, bsl], woB_sb)

            # softplus(x + bo) = ln(1 + exp(x + bo)); Softplus LUT not in sim
            eo = small.tile([128, 5 * FD], dt_f32, tag="eo")
            sp = small.tile([128, 5 * FD], dt_f32, tag="sp")
            nc.vector.tensor_tensor(eo[:], psO[:, 0:5 * FD], boT_sb, op.add)
            nc.scalar.activation(eo[:], eo[:], act.Exp, bias=0.0, scale=1.0)
            nc.scalar.activation(sp[:], eo[:], act.Ln, bias=1.0, scale=1.0)

            spR = sp[:, 0:FD]
            spG = sp[:, FD:2 * FD]
            spI0 = sp[:, 2 * FD:3 * FD]
            spK = sp[:, 3 * FD:4 * FD]
            spT = sp[:, 4 * FD:5 * FD]

            # ------------- derived params (all DVE) -------------
            gamT = small.tile([128, FD], dt_f32, tag="gamT")
            nAT = small.tile([128, FD], dt_f32, tag="nAT")
            nCT = small.tile([128, FD], dt_f32, tag="nCT")
            ntT = small.tile([128, FD], dt_f32, tag="ntT")
            rT = small.tile([128, FD], dt_f32, tag="rT")
            b0T = small.tile([128, FD], dt_f32, tag="b0T")
            Y = small.tile([128, 2 * FD], dt_f32, tag="Y")
            V = nc.vector
            V.tensor_scalar_add(gamT[:], spG, 0.1)
            V.tensor_scalar_add(rT[:], spR, 1.5)
            V.tensor_tensor(b0T[:], rT[:], gamT[:], op.mult)       # beta0
            V.tensor_scalar_add(rT[:], spK, -1.0)                  # kint-1
            V.tensor_tensor(nAT[:], b0T[:], rT[:], op.mult)
            nat_inst = V.tensor_scalar_mul(nAT[:], nAT[:], -1.0 / N_POP)
            nct_inst = V.tensor_scalar_mul(nCT[:], b0T[:], -1.0 / N_POP)
            V.tensor_scalar_add(Y[:, 0:FD], spI0, -N_POP)          # Sh0
            V.tensor_copy(Y[:, FD:2 * FD], spI0)                   # I0
            # ntT last: table build's single DVE wait covers all param tiles
            ntt_inst = V.tensor_scalar(ntT[:], spT, 20.0, -1.0 / GATE,
                                       op.add, op.mult)
            _add_dep_helper(ntt_inst.ins, nat_inst.ins, sync=True,
                            reason="ntT scheduled after nAT")
            _add_dep_helper(ntt_inst.ins, nct_inst.ins, sync=True,
                            reason="ntT scheduled after nCT")

            # ------------- beta table build (ACT) -------------
            tbl = big.tile([128, ngrid * FD], dt_f32, tag="tbl")
            tblv = tbl[:].rearrange("p (t j) -> p t j", j=FD)
            sig = big.tile([128, ngrid], dt_f32, tag="sig")
            # ordered chain: Copy-touch of ntT gives ACT the DVE clock (1
            # wait); a dummy sigmoid RAW-chained on it absorbs the ACT
            # function-table switch; the real build WAW-chains on sig.
            nc.scalar.activation(pt[:, 4:5], ntT[:, 0:1], act.Copy,
                                 bias=0.0, scale=1.0)
            nc.scalar.activation(sig[:, 0:1], pt[:, 4:5], act.Sigmoid,
                                 bias=0.0, scale=1.0)
            last_tbl_inst = None
            for j in range(FD):
                nc.scalar.activation(sig[:], tgrid_sb[:], act.Sigmoid,
                                     bias=ntT[:, j:j + 1], scale=1.0)
                last_tbl_inst = nc.scalar.activation(
                    tblv[:, :, j], sig[:], act.Identity,
                    bias=nCT[:, j:j + 1], scale=nAT[:, j:j + 1])

            # ------------- RK4 scan (single engine: DVE) -------------
            K = small.tile([128, 2 * FD], dt_f32, tag="K")
            Ysub = small.tile([128, 2 * FD], dt_f32, tag="Ysub")
            w = small.tile([128, FD], dt_f32, tag="w")
            g = small.tile([128, FD], dt_f32, tag="g")
            acc = small.tile([128, 2 * FD], dt_f32, tag="acc")

            def q(tau):  # qneg table column block [128, FD]
                return tbl[:, tau * FD:(tau + 1) * FD]

            for n in range(n_steps):
                t0, th, t1 = 2 * n, 2 * n + 1, 2 * n + 2
                Sh, Ii = Y[:, 0:FD], Y[:, FD:2 * FD]
                Ss, Is = Ysub[:, 0:FD], Ysub[:, FD:2 * FD]
                a0 = acc[:, 0:FD]
                a1 = acc[:, FD:2 * FD]
                K0, K1 = K[:, 0:FD], K[:, FD:2 * FD]

                # stage 1
                w1_inst = V.tensor_tensor(w[:], Sh, Ii, op.mult)
                if n == 0:
                    # keep the whole scan scheduled after the table build
                    _add_dep_helper(w1_inst.ins, last_tbl_inst.ins, sync=True,
                                    reason="scan after table build")
                V.tensor_tensor(a0, q(t0), w[:], op.mult)          # fl1
                g1_inst = V.tensor_tensor(g[:], gamT[:], Ii, op.mult)
                if n == 0:
                    _add_dep_helper(g1_inst.ins, last_tbl_inst.ins, sync=True,
                                    reason="scan after table build")
                V.tensor_tensor(a1, a0, g[:], op.subtract)         # kI1
                V.scalar_tensor_tensor(Ysub[:], acc[:], 0.5, Y[:], op.mult, op.add)
                # stage 2
                V.tensor_tensor(w[:], Ss, Is, op.mult)
                V.tensor_tensor(K0, q(th), w[:], op.mult)
                V.tensor_tensor(g[:], gamT[:], Is, op.mult)
                V.tensor_tensor(K1, K0, g[:], op.subtract)
                V.scalar_tensor_tensor(Ysub[:], K[:], 0.5, Y[:], op.mult, op.add)
                V.scalar_tensor_tensor(acc[:], K[:], 2.0, acc[:], op.mult, op.add)
                # stage 3
                V.tensor_tensor(w[:], Ss, Is, op.mult)
                V.tensor_tensor(K0, q(th), w[:], op.mult)
                V.tensor_tensor(g[:], gamT[:], Is, op.mult)
                V.tensor_tensor(K1, K0, g[:], op.subtract)
                V.tensor_tensor(Ysub[:], K[:], Y[:], op.add)
                V.scalar_tensor_tensor(acc[:], K[:], 2.0, acc[:], op.mult, op.add)
                # stage 4
                V.tensor_tensor(w[:], Ss, Is, op.mult)
                V.tensor_tensor(K0, q(t1), w[:], op.mult)
                V.tensor_tensor(g[:], gamT[:], Is, op.mult)
                V.tensor_tensor(K1, K0, g[:], op.subtract)
                V.tensor_tensor(acc[:], K[:], acc[:], op.add)
                # final: dI row t=n+1, then in-place Y update
                V.tensor_scalar_mul(histv[:, :, n + 1], a0, 1.0 / 6.0)
                V.scalar_tensor_tensor(Y[:], acc[:], 1.0 / 6.0, Y[:], op.mult, op.add)

            # ------------- writeback -------------
            nc.sync.dma_start(
                dI_out[:].rearrange("(p j) t -> p (j t)", p=128), hist[:])

    _split_multi_waits(nc, mybir)
    return nc


def _split_multi_waits(nc, mybir):
    """walrus in this toolchain accepts at most one sync wait per
    instruction: hoist extra waits onto same-engine NoOps placed just
    before the instruction (the engine stream executes them in order)."""
    for bb in nc.main_func.blocks:
        insts = list(bb.instructions)
        out = []
        changed = False
        for ins in insts:
            si = ins.sync_info
            if si is not None and len(si.on_wait) > 1:
                waits = list(si.on_wait)
                for wt in waits[:-1]:
                    nop = mybir.InstNoOp(
                        name=nc.get_next_instruction_name(),
                        engine=ins.engine,
                        ins=[], outs=[],
                        sync_info=mybir.SyncInfo(on_wait=[wt], on_update=[]),
                    )
                    out.append(nop)
                changed = True
                ins.sync_info = mybir.SyncInfo(on_wait=[waits[-1]],
                                               on_update=list(si.on_update))
            out.append(ins)
        if changed:
            bb.instructions = out


def _host_prep(inputs, n_steps):
    data = np.ascontiguousarray(np.asarray(inputs["data"], np.float32))
    W0 = np.asarray(inputs["W0"], np.float32)
    b0 = np.asarray(inputs["b0"], np.float32)
    Wh = np.asarray(inputs["Wh"], np.float32)
    bh = np.asarray(inputs["bh"], np.float32)
    Wo = np.asarray(inputs["Wo"], np.float32)
    bo = np.asarray(inputs["bo"], np.float32)

    ngrid = 2 * n_steps + 1
    grid = (np.arange(ngrid, dtype=np.float64) * 0.5).astype(np.float32)
    tgridG = np.ascontiguousarray(
        np.broadcast_to((grid / np.float32(GATE)).astype(np.float32),
                        (128, ngrid)))

    wpk = np.zeros((128, WPK), np.float32)
    wpk[0:D_IN, OFF_W0A:OFF_W0A + 128] = \
        W0[0:4].transpose(2, 0, 1).reshape(D_IN, 4 * H)
    wpk[0:D_IN, OFF_W0B:OFF_W0B + H] = W0[4].T
    for l in range(NH):
        for n in range(4):
            wpk[n * H:(n + 1) * H, OFF_WHA + 160 * l + n * H:
                OFF_WHA + 160 * l + (n + 1) * H] = Wh[n, l].T
        wpk[0:H, OFF_WHB + 160 * l:OFF_WHB + 160 * l + H] = Wh[4, l].T
        wpk[:, OFF_BHA + 2 * l] = bh[0:4, l].reshape(128)
        wpk[0:H, OFF_BHB + 2 * l] = bh[4, l]
    for n in range(4):
        wpk[n * H:(n + 1) * H, OFF_WO + n] = Wo[n, 0]
    wpk[0:H, OFF_WO3] = Wo[3, 0]
    wpk[0:H, OFF_WOB] = Wo[4, 0]
    wpk[:, OFF_B0A] = b0[0:4].reshape(128)
    wpk[0:H, OFF_B0B] = b0[4]
    for n in range(NN):
        wpk[:, OFF_BOT + n * FD:OFF_BOT + (n + 1) * FD] = bo[n, 0]

    in_maps = []
    for c in range(N_CORES):
        m = {"tgridG": tgridG}
        w = wpk.copy()
        w[0:D_IN, OFF_X:OFF_X + BL] = data[c * BL:(c + 1) * BL].T
        m["wpack"] = w
        in_maps.append(m)
    return in_maps


def _assemble(results, n_steps):
    nt = n_steps + 1
    full = np.empty((nt, N_CORES * BL), np.float32)
    for c in range(N_CORES):
        # device row r = p*FD + j holds batch col b = j*128 + p
        arr = results[c]["dI"].reshape(128, FD, nt).transpose(1, 0, 2)
        full[:, c * BL:(c + 1) * BL] = arr.reshape(BL, nt).T
    return full


def kernel(**inputs):
    os.environ.setdefault("JAX_PLATFORMS", "axon")
    from concourse.bass_utils import run_bass_kernel_spmd

    n_steps = NSTEP
    key = ("prog", n_steps)
    if key not in _CACHE:
        _CACHE[key] = _build_program(n_steps)
    nc = _CACHE[key]

    in_maps = _host_prep(inputs, n_steps)
    res = run_bass_kernel_spmd(nc, in_maps, list(range(N_CORES)))
    return _assemble(res.results, n_steps).reshape(-1, T)



# revision 6
# speedup vs baseline: 1.5310x; 1.5310x over previous
"""SIR-MLP RK4 scan kernel for 8 Trainium2 cores.

Math (per batch element b):
  5 small MLPs produce params R, gamma, I0, kint, tint.
  beta(t) = sigmoid((t - tint)/1.75) * (beta1 - beta0) + beta0,
  beta0 = R*gamma, beta1 = beta0*kint.
  RK4 with dt=1 over t = 0..364 of SIR:
     dS/dt = -beta*S*I/N,  dI/dt = beta*S*I/N - gamma*I
  output[t, b] = S[t-1, b] - S[t, b]  (0 for t=0), reshaped (-1, 365).

Device strategy (pure data parallel, batch sharded 8 ways, 2048/core):
  - state held negated (Sh = -S); with q(t) = -beta(t)/N from a
    precomputed [128, 729*16] table: u = [q*Sh|q*Sh] (broadcast-pair),
    v = u - [0|gamma], k = v ⊙ [I|I] = [flux | flux - gamma*I] gives a
    full RK4 stage in 3 DVE ops + 1 state-combine (19 ops/step total).
  - state+history unified: Y_n = [Sh_n|I_n] lives in a [128, 365*32]
    arena; the final combine of step n writes slot n+1 directly, so the
    S history needs no extra per-step ops. Host diffs Sh for dI.
  - batch layout on a core: column b in [0,2048) of xT; scan tiles are
    [128 partitions, 16 cols] with b = j*128 + p  (j = col, p = partition),
    which lets the MLP head write params directly into scan layout via
    one [32,128]-stationary matmul per (net, j).
  - the q-table is built by the ACT engine in time-chunks that overlap
    the DVE scan; arena slots are DMA'd to DRAM in chunks as they
    finalize, so both hide under the scan.
  - post passes strip same-engine semaphore waits (engine program order
    already serializes them), drop un-waited sem updates, and dedup
    already-observed cross-engine waits: the scan runs semaphore-free at
    the DVE issue rate instead of paying sem-completion latency per op.
"""

import os
import numpy as np

B = 16384
D_IN = 8
H = 32
T = 365
NN = 5
NH = 3
N_CORES = 8
BL = B // N_CORES          # 2048 batch per core
FD = BL // 128             # 16 batch columns per partition
N_POP = 8.6e6
GATE = 7.0 / 4.0
NSTEP = T - 1
NGRID = 2 * NSTEP + 1      # 729 half-step time points

# packed-weights column offsets in wpack [128, WPK]
OFF_W0A = 0              # [8, 128]  W0 nets 0-3, d-major
OFF_W0B = 128            # [8, 32]   W0 net 4
OFF_WHA = 160            # + 160*l   [128, 128] block-diag Wh nets 0-3
OFF_WHB = 288            # + 160*l   [32, 32]   Wh net 4
OFF_WO = 640             # [128, 4]  Wo nets 0-3 block rows
OFF_WOB = 644            # [32, 1]   Wo net 4
OFF_B0A = 645            # [128, 1]
OFF_B0B = 646            # [32, 1]
OFF_BHA = 647            # + 2*l [128, 1]
OFF_BHB = 648            # + 2*l [32, 1]
OFF_BOT = 653            # [128, 80] bo replicated in head layout
OFF_WO3 = 733            # [32, 1] Wo net 3 at base partition 0
OFF_X = 734              # [8, BL]
WPK = 734 + BL

_CACHE = {}


def _table_chunks(ngrid):
    """Grid-index ranges for the ACT table build: a small first chunk so
    the scan starts early, then larger ones that hide under the scan."""
    bounds = [0, 33]
    while bounds[-1] < ngrid:
        bounds.append(min(bounds[-1] + 116, ngrid))
    return list(zip(bounds[:-1], bounds[1:]))


def _build_program(n_steps):
    import concourse.bass as bass
    import concourse.tile as tile
    import concourse.mybir as mybir

    dt_f32 = mybir.dt.float32
    op = mybir.AluOpType
    act = mybir.ActivationFunctionType
    ngrid = 2 * n_steps + 1
    nt = n_steps + 1

    nc = bass.Bass()

    tg = nc.declare_dram_parameter("tgridG", [128, ngrid], dt_f32, isOutput=False)
    wpack = nc.declare_dram_parameter("wpack", [128, WPK], dt_f32, isOutput=False)
    sh_out = nc.declare_dram_parameter("dI", [128, nt * 2 * FD], dt_f32,
                                       isOutput=True)

    with tile.TileContext(nc) as tc:  # noqa: SIM117
        with (
            tc.tile_pool(name="big", bufs=1) as big,
            tc.tile_pool(name="mlp", bufs=2) as mlp,
            tc.tile_pool(name="wts", bufs=1) as wts,
            tc.tile_pool(name="small", bufs=1) as small,
            tc.tile_pool(name="psA", bufs=2, space="PSUM") as psA,
        ):
            # ---------------- loads ----------------
            wp = wts.tile([128, WPK], dt_f32, tag="wp")
            nc.sync.dma_start(wp[:], wpack[:])
            tgrid_sb = big.tile([128, ngrid], dt_f32, tag="tgrid")
            nc.sync.dma_start(tgrid_sb[:], tg[:])

            x_sb = wp[0:D_IN, OFF_X:OFF_X + BL]
            w0A_sb = wp[0:D_IN, OFF_W0A:OFF_W0A + 128]
            w0B_sb = wp[0:D_IN, OFF_W0B:OFF_W0B + H]
            whA_sb = [wp[:, OFF_WHA + 160 * l:OFF_WHA + 160 * l + 128]
                      for l in range(NH)]
            whB_sb = [wp[0:H, OFF_WHB + 160 * l:OFF_WHB + 160 * l + H]
                      for l in range(NH)]
            wo_sb = wp[:, OFF_WO:OFF_WO + 4]
            woB_sb = wp[0:H, OFF_WOB:OFF_WOB + 1]
            b0A_sb = wp[:, OFF_B0A:OFF_B0A + 1]
            b0B_sb = wp[0:H, OFF_B0B:OFF_B0B + 1]
            bhA_sb = [wp[:, OFF_BHA + 2 * l:OFF_BHA + 2 * l + 1] for l in range(NH)]
            bhB_sb = [wp[0:H, OFF_BHB + 2 * l:OFF_BHB + 2 * l + 1] for l in range(NH)]
            boT_sb = wp[:, OFF_BOT:OFF_BOT + 5 * FD]
            wo3_sb = wp[0:H, OFF_WO3:OFF_WO3 + 1]

            bf16 = mybir.dt.bfloat16

            # ---------------- MLP ----------------
            # nets 0-3 packed on 128 partitions; net 4 in [H, BL] tiles
            # PSUM: psmlp [128,1024]x2 (4 banks) + psB [32,1024]x2 (4 banks)
            h_prev = mlp.tile([128, BL], dt_f32, tag="h")
            hB_prev = mlp.tile([H, BL], dt_f32, tag="hB")
            for half in range(2):
                hs = slice(half * 1024, (half + 1) * 1024)
                ps = psA.tile([128, 1024], dt_f32, tag="psmlp")
                psB = psA.tile([H, 1024], dt_f32, tag="psB")
                for c in range(2):
                    sl = slice(half * 1024 + c * 512, half * 1024 + (c + 1) * 512)
                    pl = slice(c * 512, (c + 1) * 512)
                    nc.tensor.matmul(ps[:, pl], w0A_sb, x_sb[:, sl])
                    nc.tensor.matmul(psB[:, pl], w0B_sb, x_sb[:, sl])
                nc.scalar.activation(h_prev[:, hs], ps[:], act.Tanh,
                                     bias=b0A_sb, scale=1.0)
                nc.scalar.activation(hB_prev[:, hs], psB[:], act.Tanh,
                                     bias=b0B_sb, scale=1.0)

            for l in range(NH):
                h_new = mlp.tile([128, BL], dt_f32, tag="h")
                hB_new = mlp.tile([H, BL], dt_f32, tag="hB")
                for half in range(2):
                    hs = slice(half * 1024, (half + 1) * 1024)
                    psl = psA.tile([128, 1024], dt_f32, tag="psmlp")
                    pslB = psA.tile([H, 1024], dt_f32, tag="psB")
                    for c in range(2):
                        sl = slice(half * 1024 + c * 512,
                                   half * 1024 + (c + 1) * 512)
                        pl = slice(c * 512, (c + 1) * 512)
                        nc.tensor.matmul(psl[:, pl], whA_sb[l], h_prev[:, sl])
                        nc.tensor.matmul(pslB[:, pl], whB_sb[l], hB_prev[:, sl])
                    nc.scalar.activation(h_new[:, hs], psl[:], act.Tanh,
                                         bias=bhA_sb[l], scale=1.0)
                    nc.scalar.activation(hB_new[:, hs], pslB[:], act.Tanh,
                                         bias=bhB_sb[l], scale=1.0)
                h_prev, hB_prev = h_new, hB_new

            # ------------- head: params straight into scan layout -------------
            # psO[:, n*FD + j] = Wo[n] . h[net n, batch col block j]
            # (one [32,128]-stationary matmul per (net, j); b = j*128 + p)
            psO = psA.tile([128, 1024], dt_f32, tag="psmlp")
            # PE operands may only start at partition 0/32/64: net 3 (base 96)
            # is copied down to a base-0 tile first
            h3 = mlp.tile([H, BL], dt_f32, tag="h3")
            nc.vector.tensor_copy(h3[:], h_prev[96:128, :])
            nc.tensor.ldweights(h_prev[0:1, 0:2].bitcast(bf16))
            nc.tensor.ldweights(hB_prev[0:1, 0:2].bitcast(bf16))
            nc.tensor.ldweights(h3[0:1, 0:2].bitcast(bf16))
            for j in range(FD):
                bsl = slice(j * 128, (j + 1) * 128)
                for n in range(3):
                    nc.tensor.matmul(psO[:, n * FD + j:n * FD + j + 1],
                                     h_prev[n * H:(n + 1) * H, bsl],
                                     wo_sb[n * H:(n + 1) * H, n:n + 1])
                nc.tensor.matmul(psO[:, 3 * FD + j:3 * FD + j + 1],
                                 h3[:, bsl], wo3_sb)
                nc.tensor.matmul(psO[:, 4 * FD + j:4 * FD + j + 1],
                                 hB_prev[:, bsl], woB_sb)

            # softplus(x + bo) = ln(1 + exp(x + bo)); Softplus LUT not in sim
            eo = small.tile([128, 5 * FD], dt_f32, tag="eo")
            sp = small.tile([128, 5 * FD], dt_f32, tag="sp")
            nc.vector.tensor_tensor(eo[:], psO[:, 0:5 * FD], boT_sb, op.add)
            nc.scalar.activation(eo[:], eo[:], act.Exp, bias=0.0, scale=1.0)
            nc.scalar.activation(sp[:], eo[:], act.Ln, bias=1.0, scale=1.0)

            spR = sp[:, 0:FD]
            spG = sp[:, FD:2 * FD]
            spI0 = sp[:, 2 * FD:3 * FD]
            spK = sp[:, 3 * FD:4 * FD]
            spT = sp[:, 4 * FD:5 * FD]

            # ------------- derived params (all DVE) -------------
            gamT = small.tile([128, FD], dt_f32, tag="gamT")
            nAT = small.tile([128, FD], dt_f32, tag="nAT")
            nCT = small.tile([128, FD], dt_f32, tag="nCT")
            ntT = small.tile([128, FD], dt_f32, tag="ntT")
            rT = small.tile([128, FD], dt_f32, tag="rT")
            b0T = small.tile([128, FD], dt_f32, tag="b0T")
            ZG = small.tile([128, 2 * FD], dt_f32, tag="ZG")
            V = nc.vector
            V.tensor_scalar_add(gamT[:], spG, 0.1)
            V.tensor_scalar_add(rT[:], spR, 1.5)
            V.tensor_tensor(b0T[:], rT[:], gamT[:], op.mult)       # beta0
            V.tensor_scalar_add(rT[:], spK, -1.0)                  # kint-1
            V.tensor_tensor(nAT[:], b0T[:], rT[:], op.mult)
            V.tensor_scalar_mul(nAT[:], nAT[:], -1.0 / N_POP)
            V.tensor_scalar_mul(nCT[:], b0T[:], -1.0 / N_POP)
            V.tensor_scalar(ntT[:], spT, 20.0, -1.0 / GATE, op.add, op.mult)
            V.memset(ZG[:], 0.0)
            V.tensor_copy(ZG[:, FD:2 * FD], gamT[:])

            # state/history arena: slot n = [Sh_n | I_n], Sh = -S
            arena = big.tile([128, nt * 2 * FD], dt_f32, tag="arena")
            V.tensor_scalar_add(arena[:, 0:FD], spI0, -N_POP)      # Sh0
            V.tensor_copy(arena[:, FD:2 * FD], spI0)               # I0

            # ------------- beta table (ACT), built in time chunks -------------
            # q(t) = -beta(t)/N laid out [p, (t j)]; chunk c is ready long
            # before the scan steps that read it (cross-engine dep, auto).
            tbl = big.tile([128, ngrid * FD], dt_f32, tag="tbl")
            tblv = tbl[:].rearrange("p (t j) -> p t j", j=FD)
            sig = small.tile([128, 116], dt_f32, tag="sig")
            for (g0, g1) in _table_chunks(ngrid):
                w = g1 - g0
                for j in range(FD):
                    nc.scalar.activation(sig[:, 0:w], tgrid_sb[:, g0:g1],
                                         act.Sigmoid, bias=ntT[:, j:j + 1],
                                         scale=1.0)
                    nc.scalar.activation(tblv[:, g0:g1, j], sig[:, 0:w],
                                         act.Identity, bias=nCT[:, j:j + 1],
                                         scale=nAT[:, j:j + 1])

            # ------------- RK4 scan (single engine: DVE) -------------
            u = small.tile([128, 2 * FD], dt_f32, tag="u")
            v = small.tile([128, 2 * FD], dt_f32, tag="v")
            k1 = small.tile([128, 2 * FD], dt_f32, tag="k1")
            k2 = small.tile([128, 2 * FD], dt_f32, tag="k2")
            k3 = small.tile([128, 2 * FD], dt_f32, tag="k3")
            k4 = small.tile([128, 2 * FD], dt_f32, tag="k4")
            ys = small.tile([128, 2 * FD], dt_f32, tag="ys")
            c1 = small.tile([128, 2 * FD], dt_f32, tag="c1")

            def bc2(ap16):  # [128,16] -> [128,2,16] stride-0 pair broadcast
                return ap16.unsqueeze(1).broadcast_to([128, 2, FD])

            def qb(g):      # q-table block at grid index g, pair-broadcast
                return bc2(tbl[:, g * FD:(g + 1) * FD])

            def pr(t32):    # view [128,32] as [128,2,16] to match bc2 rank
                return t32.rearrange("p (a c) -> p a c", a=2)

            # DMA the arena out in slot chunks as they finalize
            dma_bounds = [0, 74, 147, 220, 293, nt]
            dma_next = 1

            for n in range(n_steps):
                g0, gh, g1 = 2 * n, 2 * n + 1, 2 * n + 2
                Yn = arena[:, 32 * n:32 * n + 32]
                YnS = arena[:, 32 * n:32 * n + 16]
                YnI = arena[:, 32 * n + 16:32 * n + 32]
                Yp = arena[:, 32 * (n + 1):32 * (n + 1) + 32]
                ysS = ys[:, 0:FD]
                ysI = ys[:, FD:2 * FD]
                # stage 1: k1 = [flux | flux - g*I] at (t0, Yn)
                V.tensor_tensor(pr(u[:]), qb(g0), bc2(YnS), op.mult)
                V.tensor_tensor(v[:], u[:], ZG[:], op.subtract)
                V.tensor_tensor(pr(k1[:]), pr(v[:]), bc2(YnI), op.mult)
                V.scalar_tensor_tensor(ys[:], k1[:], 0.5, Yn, op.mult, op.add)
                # stage 2
                V.tensor_tensor(pr(u[:]), qb(gh), bc2(ysS), op.mult)
                V.tensor_tensor(v[:], u[:], ZG[:], op.subtract)
                V.tensor_tensor(pr(k2[:]), pr(v[:]), bc2(ysI), op.mult)
                V.scalar_tensor_tensor(ys[:], k2[:], 0.5, Yn, op.mult, op.add)
                # stage 3
                V.tensor_tensor(pr(u[:]), qb(gh), bc2(ysS), op.mult)
                V.tensor_tensor(v[:], u[:], ZG[:], op.subtract)
                V.tensor_tensor(pr(k3[:]), pr(v[:]), bc2(ysI), op.mult)
                V.tensor_tensor(ys[:], k3[:], Yn, op.add)
                # stage 4
                V.tensor_tensor(pr(u[:]), qb(g1), bc2(ysS), op.mult)
                V.tensor_tensor(v[:], u[:], ZG[:], op.subtract)
                V.tensor_tensor(pr(k4[:]), pr(v[:]), bc2(ysI), op.mult)
                # combine: Y_{n+1} = Yn + (k1 + 2(k2+k3) + k4)/6 -> slot n+1
                V.tensor_tensor(c1[:], k2[:], k3[:], op.add)
                V.scalar_tensor_tensor(c1[:], c1[:], 2.0, k1[:], op.mult, op.add)
                V.tensor_tensor(c1[:], c1[:], k4[:], op.add)
                V.scalar_tensor_tensor(Yp, c1[:], 1.0 / 6.0, Yn, op.mult, op.add)

                if dma_next < len(dma_bounds) and n + 2 == dma_bounds[dma_next]:
                    a = dma_bounds[dma_next - 1] * 2 * FD
                    b = dma_bounds[dma_next] * 2 * FD
                    nc.sync.dma_start(sh_out[:, a:b], arena[:, a:b])
                    dma_next += 1

    _strip_same_engine_waits(nc, mybir)
    _dedup_cross_waits(nc, mybir)
    _split_multi_waits(nc, mybir)
    return nc


_COMPUTE_INSTS = {
    "InstTensorTensor", "InstTensorScalarPtr", "InstTensorScalar",
    "InstTensorReduce", "InstActivation", "InstMatmult", "InstLdweights",
    "InstMemset", "InstTensorCopy", "InstCopy", "InstDrain",
    "InstEventSemaphore", "InstNoOp", "InstRegisterMove", "InstSelect",
    "InstIota", "InstRegisterAlu",
}


def _sem_usage(nc):
    """Map sem id -> (waiter entries, updater entries) across all blocks."""
    use = {}
    for f in nc.m.functions:
        for bb in f.blocks:
            for ins in bb.instructions:
                si = ins.sync_info
                if si is None:
                    continue
                for w in si.on_wait:
                    use.setdefault(w.id, ([], []))[0].append((ins, w))
                for u in si.on_update:
                    use.setdefault(u.id, ([], []))[1].append((ins, u))
    return use


def _qualifying_sems(nc):
    """Sems where every updater is a compute instruction on ONE engine,
    all updates are sem-inc by 1, and all waits are sem-ge-imm: for these,
    a same-engine wait is subsumed by engine program order."""
    out = {}
    for sem_id, (waits, upds) in _sem_usage(nc).items():
        if not upds:
            continue
        engs = {str(i.engine) for i, _ in upds}
        if len(engs) != 1:
            continue
        if not all(type(i).__name__ in _COMPUTE_INSTS for i, _ in upds):
            continue
        if not all(u.update_mode == "sem-inc" and u.update_value == 1
                   and u.update_reg is None for _, u in upds):
            continue
        if not all(w.wait_mode == "sem-ge-imm" and w.wait_reg is None
                   for _, w in waits):
            continue
        out[sem_id] = engs.pop()
    return out


def _strip_same_engine_waits(nc, mybir):
    """Remove waits on single-engine monotonic sems when the waiting
    instruction runs on that same engine: its program order already
    guarantees every earlier same-engine instruction has completed.

    Then drop sem updates nobody waits for: keep an update only at the
    positions (cumulative counts) some remaining wait references, bumping
    update_value to cover the skipped increments."""
    qual = _qualifying_sems(nc)
    if not qual:
        return
    # pass 1: strip same-engine waits
    for f in nc.m.functions:
        for bb in f.blocks:
            for ins in bb.instructions:
                si = ins.sync_info
                if si is None or not si.on_wait:
                    continue
                eng = str(ins.engine)
                kept = [w for w in si.on_wait
                        if not (w.id in qual and qual[w.id] == eng)]
                if len(kept) != len(si.on_wait):
                    ins.sync_info = mybir.SyncInfo(
                        on_wait=kept, on_update=list(si.on_update))
    # pass 2: drop updates nobody waits on. walrus requires update_value
    # == 1, so keep unit updates exactly at the waited cumulative counts
    # and remap each wait value to the RANK of its threshold.
    use = _sem_usage(nc)
    for sem_id, eng in qual.items():
        waits, upds = use.get(sem_id, ([], []))
        thresholds = sorted({w.wait_value for _, w in waits if w.wait_value >= 1})
        rank = {v: i + 1 for i, v in enumerate(thresholds)}
        for ins, w in waits:
            if w.wait_value >= 1:
                w.wait_value = rank[w.wait_value]
        pos = 0
        ti = 0
        for ins, u in upds:
            pos += 1
            keep = ti < len(thresholds) and pos == thresholds[ti]
            si = ins.sync_info
            if keep:
                ti += 1
            else:
                new_upds = [uu for uu in si.on_update if uu.id != sem_id]
                if len(new_upds) != len(si.on_update):
                    ins.sync_info = mybir.SyncInfo(
                        on_wait=list(si.on_wait), on_update=new_upds)


def _dedup_cross_waits(nc, mybir):
    """Drop waits already implied by an earlier wait on the same engine:
    once engine E observed monotonic sem >= v, every later E instruction
    inherits that bound through program order."""
    qual = _qualifying_sems(nc)
    for f in nc.m.functions:
        seen = {}
        for bb in f.blocks:
            for ins in bb.instructions:
                si = ins.sync_info
                if si is None or not si.on_wait:
                    continue
                eng = str(ins.engine)
                kept = []
                for w in si.on_wait:
                    if w.id in qual:
                        prev = seen.get((eng, w.id), -1)
                        if w.wait_value <= prev:
                            continue
                        seen[(eng, w.id)] = w.wait_value
                    kept.append(w)
                if len(kept) != len(si.on_wait):
                    ins.sync_info = mybir.SyncInfo(
                        on_wait=kept, on_update=list(si.on_update))


def _split_multi_waits(nc, mybir):
    """walrus in this toolchain accepts at most one sync wait per
    instruction: hoist extra waits onto same-engine NoOps placed just
    before the instruction (the engine stream executes them in order)."""
    for f in nc.m.functions:
        for bb in f.blocks:
            insts = list(bb.instructions)
            out = []
            changed = False
            for ins in insts:
                si = ins.sync_info
                if si is not None and len(si.on_wait) > 1:
                    waits = list(si.on_wait)
                    for wt in waits[:-1]:
                        nop = mybir.InstNoOp(
                            name=nc.get_next_instruction_name(),
                            engine=ins.engine,
                            ins=[], outs=[],
                            sync_info=mybir.SyncInfo(on_wait=[wt], on_update=[]),
                        )
                        out.append(nop)
                    changed = True
                    ins.sync_info = mybir.SyncInfo(on_wait=[waits[-1]],
                                                   on_update=list(si.on_update))
                out.append(ins)
            if changed:
                bb.instructions = out


def _host_prep(inputs, n_steps):
    data = np.ascontiguousarray(np.asarray(inputs["data"], np.float32))
    W0 = np.asarray(inputs["W0"], np.float32)
    b0 = np.asarray(inputs["b0"], np.float32)
    Wh = np.asarray(inputs["Wh"], np.float32)
    bh = np.asarray(inputs["bh"], np.float32)
    Wo = np.asarray(inputs["Wo"], np.float32)
    bo = np.asarray(inputs["bo"], np.float32)

    ngrid = 2 * n_steps + 1
    grid = (np.arange(ngrid, dtype=np.float64) * 0.5).astype(np.float32)
    tgridG = np.ascontiguousarray(
        np.broadcast_to((grid / np.float32(GATE)).astype(np.float32),
                        (128, ngrid)))

    wpk = np.zeros((128, WPK), np.float32)
    wpk[0:D_IN, OFF_W0A:OFF_W0A + 128] = \
        W0[0:4].transpose(2, 0, 1).reshape(D_IN, 4 * H)
    wpk[0:D_IN, OFF_W0B:OFF_W0B + H] = W0[4].T
    for l in range(NH):
        for n in range(4):
            wpk[n * H:(n + 1) * H, OFF_WHA + 160 * l + n * H:
                OFF_WHA + 160 * l + (n + 1) * H] = Wh[n, l].T
        wpk[0:H, OFF_WHB + 160 * l:OFF_WHB + 160 * l + H] = Wh[4, l].T
        wpk[:, OFF_BHA + 2 * l] = bh[0:4, l].reshape(128)
        wpk[0:H, OFF_BHB + 2 * l] = bh[4, l]
    for n in range(4):
        wpk[n * H:(n + 1) * H, OFF_WO + n] = Wo[n, 0]
    wpk[0:H, OFF_WO3] = Wo[3, 0]
    wpk[0:H, OFF_WOB] = Wo[4, 0]
    wpk[:, OFF_B0A] = b0[0:4].reshape(128)
    wpk[0:H, OFF_B0B] = b0[4]
    for n in range(NN):
        wpk[:, OFF_BOT + n * FD:OFF_BOT + (n + 1) * FD] = bo[n, 0]

    in_maps = []
    for c in range(N_CORES):
        m = {"tgridG": tgridG}
        w = wpk.copy()
        w[0:D_IN, OFF_X:OFF_X + BL] = data[c * BL:(c + 1) * BL].T
        m["wpack"] = w
        in_maps.append(m)
    return in_maps


def _assemble(results, n_steps):
    nt = n_steps + 1
    full = np.empty((nt, N_CORES * BL), np.float32)
    for c in range(N_CORES):
        arr = results[c]["dI"].reshape(128, nt, 2 * FD)
        sh = arr[:, :, 0:FD]                       # Sh[p, t, j], Sh = -S
        dsh = np.concatenate(
            [np.zeros((128, 1, FD), np.float32), sh[:, 1:] - sh[:, :-1]],
            axis=1)                                # dI[t] = Sh[t] - Sh[t-1]
        # batch col b = j*128 + p
        full[:, c * BL:(c + 1) * BL] = dsh.transpose(1, 2, 0).reshape(nt, BL)
    return full


def kernel(**inputs):
    os.environ.setdefault("JAX_PLATFORMS", "axon")
    from concourse.bass_utils import run_bass_kernel_spmd

    n_steps = NSTEP
    key = ("prog", n_steps)
    if key not in _CACHE:
        _CACHE[key] = _build_program(n_steps)
    nc = _CACHE[key]

    in_maps = _host_prep(inputs, n_steps)
    res = run_bass_kernel_spmd(nc, in_maps, list(range(N_CORES)))
    return _assemble(res.results, n_steps).reshape(-1, T)
